# revision 1
# baseline (speedup 1.0000x reference)
"""Signature-kernel Gram matrix on 8 NeuronCores.

Math (per pair of sequences x (128,8), y (128,8)):
  K = exp(x@y.T - 0.5|x|^2 - 0.5|y|^2)            (RBF gram, sigma=1)
  diff = second mixed finite difference of K       (127,127)
  inc  = diff/4 on a dyadic-refined (254,254) grid (2x2 constant blocks)
  Goursat PDE grid G (255,255), G[0,:]=G[:,0]=1,
    G[i,j] = c1*(G[i-1,j]+G[i,j-1]) - c2*G[i-1,j-1]
    c1 = 1 + inc/2 + inc^2/12 = 1 + diff/8 + diff^2/192
    c2 = 1 - inc^2/12         = 1 - diff^2/192
  answer = G[254,254]

Row-sweep formulation: along each grid row i,
    G[i,j] = c1[j]*G[i,j-1] + (c1[j]*G[i-1,j] - c2[j]*G[i-1,j-1])
is a first-order linear recurrence in j -> one DVE tensor_tensor_scan
(op0=mult, op1=add, initial=1.0) per row, plus 3 elementwise ops to
build the additive term. 254 rows, 32 pairs per core on 32 partitions.

Sharding: data-parallel over batch_x: core c owns x rows {2c, 2c+1} x all
16 ys = 32 pairs. Host gathers the (16,16) output.
"""

import numpy as np
from contextlib import ExitStack

import concourse.bass as bass
import concourse.bacc as bacc
import concourse.tile as tile
from concourse import mybir
from concourse.bass_utils import run_bass_kernel_spmd

F32 = mybir.dt.float32
AL = mybir.AluOpType
AF = mybir.ActivationFunctionType

N_CORES = 8
L = 128          # sequence length
D = 8            # feature dim
NY = 16          # all ys per core
NX = 2           # xs per core
NP = NX * NY     # 32 pairs per core
M = L - 1        # 127 coarse grid
G = 2 * M        # 254 fine grid (dyadic order 1)


def _repeat2(ap):
    """View a [P, n] AP as [P, n, 2] with zero-stride inner dim (each
    element read twice consecutively) -> free sequence of length 2n."""
    return bass.AP(
        tensor=ap.tensor,
        offset=ap.offset,
        ap=[ap.ap[0], ap.ap[1], [0, 2]],
    )


def _build():
    nc = bacc.Bacc()
    xs_t = nc.dram_tensor("xs", [NX * L, D], F32, kind="ExternalInput")
    ys_t = nc.dram_tensor("ys", [NY * L, D], F32, kind="ExternalInput")
    idn_t = nc.dram_tensor("idn", [L, L], F32, kind="ExternalInput")
    shf_t = nc.dram_tensor("shf", [L, L], F32, kind="ExternalInput")
    out_t = nc.dram_tensor("out", [NP, 1], F32, kind="ExternalOutput")

    NSEQ = NX + NY

    with ExitStack() as ctx:
        tc = ctx.enter_context(tile.TileContext(nc))
        constp = ctx.enter_context(tc.tile_pool(name="constp", bufs=1))
        iop = ctx.enter_context(tc.tile_pool(name="iop", bufs=3))
        psp = ctx.enter_context(tc.tile_pool(name="psp", bufs=2, space="PSUM"))
        workp = ctx.enter_context(tc.tile_pool(name="workp", bufs=3))
        cbp = ctx.enter_context(tc.tile_pool(name="cbp", bufs=1))
        bigp = ctx.enter_context(tc.tile_pool(name="bigp", bufs=1))
        rowp = ctx.enter_context(tc.tile_pool(name="rowp", bufs=4))
        dramp = ctx.enter_context(tc.tile_pool(name="dramp", bufs=1, space="DRAM"))

        # Stage DMA-loaded constants through a DVE copy so PE matmuls never
        # wait directly on DMA-queue semaphores (codegen rejects a PE op
        # with two DMA-HW waits: "Too many sync wait commands").
        idn_s = iop.tile([L, L], F32, tag="idn_s")
        nc.sync.dma_start(out=idn_s, in_=idn_t[:, :])
        idn = constp.tile([L, L], F32)
        nc.vector.tensor_copy(idn, idn_s)
        shf_s = iop.tile([L, L], F32, tag="shf_s")
        nc.sync.dma_start(out=shf_s, in_=shf_t[:, :])
        shf = constp.tile([L, L], F32)
        nc.vector.tensor_copy(shf, shf_s)
        ones8 = constp.tile([D, 1], F32)
        nc.vector.memset(ones8, 1.0)

        # ---- Phase A: transposed sequences + norm rows ----
        # AUG[:, s*128:(s+1)*128] = seq^T (8 rows);  NRM[0, s*128+a] =
        # -0.5|seq_a|^2 ; ONE = row of ones.  The RBF exponent
        # x.y - 0.5|x|^2 - 0.5|y|^2 is built by 3 accumulating matmuls so
        # every operand starts at partition 0 (HW alignment rule).
        AUG = constp.tile([D, NSEQ * L], F32)
        NRM = constp.tile([1, NSEQ * L], F32)
        ONE = constp.tile([1, L], F32)
        nc.vector.memset(ONE, 1.0)
        for s in range(NSEQ):
            if s < NY:
                src = ys_t[s * L : (s + 1) * L, :]
            else:
                src = xs_t[(s - NY) * L : (s - NY + 1) * L, :]
            raw_s = iop.tile([L, D], F32, tag="raw_s", bufs=NSEQ)
            nc.sync.dma_start(out=raw_s, in_=src)
            raw = iop.tile([L, D], F32, tag="raw", bufs=NSEQ)
            nc.vector.tensor_copy(raw, raw_s)
            pst = psp.tile([D, L], F32, tag="pst")
            nc.tensor.transpose(pst, raw, idn)
            nc.scalar.activation(AUG[0:D, s * L : (s + 1) * L], pst, AF.Copy)
            sq = workp.tile([D, L], F32, tag="sq")
            nc.scalar.square(sq, pst)
            nrm = psp.tile([1, L], F32, tag="nrm")
            nc.tensor.matmul(nrm, ones8, sq)
            nc.scalar.activation(
                NRM[0:1, s * L : (s + 1) * L], nrm, AF.Copy, scale=-0.5
            )

        # ---- Phases B+C+D per half (16 pairs) to bound SBUF ----
        HP = NP // 2  # 16 pairs per half
        # Final coefficient arrays: pair-per-partition, coarse (ic, jc) flat.
        C1F = bigp.tile([NP, M * M], F32)
        C2F = bigp.tile([NP, M * M], F32)

        for h in range(2):
            DIFFB = cbp.tile([M, HP * M], F32, tag="a")
            for pl in range(HP):
                p = h * HP + pl
                iloc, j = p // NY, p % NY
                xsl = slice((NY + iloc) * L, (NY + iloc + 1) * L)
                ysl = slice(j * L, (j + 1) * L)
                kps = psp.tile([L, L], F32, tag="kps")
                nc.tensor.matmul(
                    kps, AUG[:, xsl], AUG[:, ysl], start=True, stop=False
                )
                nc.tensor.matmul(
                    kps, NRM[:, xsl], ONE, start=False, stop=False
                )
                nc.tensor.matmul(
                    kps, ONE, NRM[:, ysl], start=False, stop=True
                )
                kex = workp.tile([L, L], F32, tag="kex")
                nc.scalar.activation(kex, kps, AF.Exp)
                # column diff along free dim
                db = workp.tile([L, M], F32, tag="db")
                nc.vector.tensor_sub(db, kex[:, 1:L], kex[:, 0:M])
                # row shift via PE: dbs[a,:] = db[a+1,:]
                dbs = psp.tile([L, M], F32, tag="dbs")
                nc.tensor.matmul(dbs, shf, db)
                nc.vector.tensor_sub(
                    DIFFB[:, pl * M : (pl + 1) * M], dbs[0:M, :], db[0:M, :]
                )
            # coefficient build (bulk):
            # QB = DIFFB^2 ; T1 = QB/192 + 1
            # c1c = DIFFB/8 + T1 ; c2c = 2 - T1
            QB = cbp.tile([M, HP * M], F32, tag="b")
            nc.scalar.square(QB, DIFFB)
            T1 = cbp.tile([M, HP * M], F32, tag="c")
            nc.scalar.activation(T1, QB, AF.Copy, bias=1.0, scale=1.0 / 192.0)
            c1c = cbp.tile([M, HP * M], F32, tag="b")
            nc.vector.scalar_tensor_tensor(c1c, DIFFB, 0.125, T1, AL.mult, AL.add)
            c2c = cbp.tile([M, HP * M], F32, tag="a")
            nc.scalar.activation(c2c, T1, AF.Copy, bias=2.0, scale=-1.0)
            # flatten: per pair, [127 partitions, 127] -> [1 partition, 16129].
            # A direct SBUF->SBUF partition->free flatten explodes into
            # per-element indirect loads in codegen, so bounce through DRAM:
            # partition-major store (native) then one contiguous load.
            for pl in range(HP):
                p = h * HP + pl
                c1dr = dramp.tile([1, M * M], F32, tag="c1dr", bufs=2 * HP)
                c2dr = dramp.tile([1, M * M], F32, tag="c2dr", bufs=2 * HP)
                # store on the ACT HWDGE ring, load on the SP ring, so each
                # DMA carries at most one semaphore wait.
                nc.scalar.dma_start(
                    out=c1dr.rearrange("o (a b) -> (o a) b", b=M),
                    in_=c1c[0:M, pl * M : (pl + 1) * M],
                )
                nc.sync.dma_start(
                    out=c2dr.rearrange("o (a b) -> (o a) b", b=M),
                    in_=c2c[0:M, pl * M : (pl + 1) * M],
                )
                nc.sync.dma_start(out=C1F[p : p + 1, :], in_=c1dr)
                nc.scalar.dma_start(out=C2F[p : p + 1, :], in_=c2dr)

        # ---- Phase E: 254 row sweeps ----
        KA = constp.tile([NP, G + 1], F32)
        KB = constp.tile([NP, G + 1], F32)
        nc.vector.memset(KA[:, :], 1.0)   # grid row 0 = 1
        nc.vector.memset(KB[:, 0:1], 1.0)  # j=0 boundary

        cur, prv = KB, KA
        for i in range(1, G + 1):
            ic = (i - 1) // 2
            c1v = _repeat2(C1F[:, ic * M : (ic + 1) * M])  # [32,127,2]
            c2v = _repeat2(C2F[:, ic * M : (ic + 1) * M])
            # materialize c1 row (2D) for the scan's data0 on ScalarE
            c12 = rowp.tile([NP, 2, G], F32, tag="c1row")
            nc.scalar.activation(
                c12[:, 0, :].rearrange("p (a b) -> p a b", b=2), c1v, AF.Copy
            )
            nc.scalar.activation(
                c12[:, 1, :].rearrange("p (a b) -> p a b", b=2), c2v, AF.Copy
            )
            # one fused DVE mul over [c1|c2] x [Kp(1:) | Kp(:-1)] via a
            # negative-stride outer dim reading prv at offsets 1 and 0
            kview = prv[:, 1 : G + 1]
            kcat = bass.AP(
                tensor=kview.tensor, offset=kview.offset,
                ap=[kview.ap[0], [-1, 2], [1, G]],
            )
            P = rowp.tile([NP, 2, G], F32, tag="t1")
            nc.vector.tensor_mul(P, c12, kcat)
            d1 = rowp.tile([NP, G], F32, tag="d1")
            nc.vector.tensor_sub(d1, P[:, 0, :], P[:, 1, :])
            nc.vector.tensor_tensor_scan(
                cur[:, 1 : G + 1], c12[:, 0, :], d1, 1.0, AL.mult, AL.add
            )
            cur, prv = prv, cur

        nc.sync.dma_start(out=out_t[:, :], in_=prv[:, G : G + 1])

    nc.finalize()
    return nc


_CACHE = {}


def _get_nc():
    if "nc" not in _CACHE:
        _CACHE["nc"] = _build()
    return _CACHE["nc"]


def run(xs, ys, trace=False):
    xs = np.ascontiguousarray(np.asarray(xs), dtype=np.float32)
    ys = np.ascontiguousarray(np.asarray(ys), dtype=np.float32)
    assert xs.shape == (16, L, D) and ys.shape == (16, L, D)
    nc = _get_nc()
    idn = np.eye(L, dtype=np.float32)
    shf = np.eye(L, k=-1, dtype=np.float32)  # shf[k,m]=1 iff k=m+1
    in_maps = []
    for c in range(N_CORES):
        in_maps.append(
            {
                "xs": xs[2 * c : 2 * c + 2].reshape(NX * L, D).copy(),
                "ys": ys.reshape(NY * L, D).copy(),
                "idn": idn,
                "shf": shf,
            }
        )
    try:
        res = run_bass_kernel_spmd(
            nc, in_maps, list(range(N_CORES)), trace=trace
        )
    except ModuleNotFoundError:
        res = run_bass_kernel_spmd(
            nc, in_maps, list(range(N_CORES)), trace=False
        )
    rows = [res.results[c]["out"].reshape(NX, NY) for c in range(N_CORES)]
    out = np.concatenate(rows, axis=0)
    return out, res


def kernel(xs, ys):
    out, _ = run(xs, ys)
    return out



# revision 4
# speedup vs baseline: 1.3118x; 1.3118x over previous
"""Signature-kernel Gram matrix on 8 NeuronCores.

Math (per pair of sequences x (128,8), y (128,8)):
  K = exp(x@y.T - 0.5|x|^2 - 0.5|y|^2)            (RBF gram, sigma=1)
  diff = second mixed finite difference of K       (127,127)
  Goursat PDE grid G (255,255), G[0,:]=G[:,0]=1,
    G[i,j] = c1*(G[i-1,j]+G[i,j-1]) - c2*G[i-1,j-1]
    c1 = 1 + diff/8 + diff^2/192,  c2 = 1 - diff^2/192
  (dyadic order 1: each coarse cell repeats 2x2 on the 254x254 fine grid)
  answer = G[254,254]

Row-sweep formulation with a single fused scan per fine row:
    y_j = c1_j*(y_{j-1} + u_j),   u_j = prv_j - r'_j*prv_{j-1},
    r' = c2/c1 (precomputed)
-> per row: DVE mult (repeat-view, no materialized coefficient row),
   DVE sub, DVE tensor_tensor_scan(op0=add, op1=mult). All three on DVE,
   no cross-engine dependency on the critical path; ACT expands the
   per-coarse-row c1 multiplier (scan data1 must be a 2-D AP) ahead of
   the sweep.

Coefficients are built bulk in [x-row partitions, pair*col free] layout,
then moved to [pair partitions, row*col free] via one padded pair-major
DRAM round trip (512B-aligned descriptor runs, 4 large DMAs total).

Sharding: data-parallel over batch_x: core c owns x rows {2c, 2c+1} x all
16 ys = 32 pairs. Host gathers the (16,16) output.
"""

import numpy as np
from contextlib import ExitStack

import concourse.bass as bass
import concourse.bacc as bacc
import concourse.tile as tile
from concourse import mybir
from concourse.bass_utils import run_bass_kernel_spmd

F32 = mybir.dt.float32
AL = mybir.AluOpType
AF = mybir.ActivationFunctionType

N_CORES = 8
L = 128          # sequence length
D = 8            # feature dim
NY = 16          # all ys per core
NX = 2           # xs per core
NP = NX * NY     # 32 pairs per core
M = L - 1        # 127 coarse grid
MP = 128         # padded coarse columns (512B DMA runs)
G = 2 * M        # 254 fine grid (dyadic order 1)
HP = NP // 2     # 16 pairs per phase-B/C half


def _rep2(ap):
    """View a [P, n] AP as [P, n, 2] with zero-stride inner dim (each
    element read twice consecutively)."""
    return bass.AP(
        tensor=ap.tensor,
        offset=ap.offset,
        ap=[ap.ap[0], ap.ap[1], [0, 2]],
    )


def _build():
    nc = bacc.Bacc()
    xs_t = nc.dram_tensor("xs", [NX * L, D], F32, kind="ExternalInput")
    ys_t = nc.dram_tensor("ys", [NY * L, D], F32, kind="ExternalInput")
    idn_t = nc.dram_tensor("idn", [L, L], F32, kind="ExternalInput")
    shf_t = nc.dram_tensor("shf", [L, L], F32, kind="ExternalInput")
    out_t = nc.dram_tensor("out", [NP, 1], F32, kind="ExternalOutput")

    NSEQ = NX + NY

    with ExitStack() as ctx:
        tc = ctx.enter_context(tile.TileContext(nc))
        constp = ctx.enter_context(tc.tile_pool(name="constp", bufs=1))
        iop = ctx.enter_context(tc.tile_pool(name="iop", bufs=3))
        psp = ctx.enter_context(tc.tile_pool(name="psp", bufs=2, space="PSUM"))
        workp = ctx.enter_context(tc.tile_pool(name="workp", bufs=3))
        cbp = ctx.enter_context(tc.tile_pool(name="cbp", bufs=1))
        bigp = ctx.enter_context(tc.tile_pool(name="bigp", bufs=1))
        rowp = ctx.enter_context(tc.tile_pool(name="rowp", bufs=4))
        c1wp = ctx.enter_context(tc.tile_pool(name="c1wp", bufs=1))
        dramp = ctx.enter_context(tc.tile_pool(name="dramp", bufs=1, space="DRAM"))

        # Stage DMA-loaded constants through a DVE copy so PE matmuls never
        # wait directly on DMA-queue semaphores (codegen rejects a PE op
        # with two DMA-HW waits).
        idn_s = iop.tile([L, L], F32, tag="idn_s")
        nc.sync.dma_start(out=idn_s, in_=idn_t[:, :])
        idn = constp.tile([L, L], F32)
        nc.vector.tensor_copy(idn, idn_s)
        shf_s = iop.tile([L, L], F32, tag="shf_s")
        nc.sync.dma_start(out=shf_s, in_=shf_t[:, :])
        shf = constp.tile([L, L], F32)
        nc.vector.tensor_copy(shf, shf_s)
        ones8 = constp.tile([D, 1], F32)
        nc.vector.memset(ones8, 1.0)

        # ---- Phase A: transposed sequences + norm rows ----
        AUG = constp.tile([D, NSEQ * L], F32)
        NRM = constp.tile([1, NSEQ * L], F32)
        ONE = constp.tile([1, L], F32)
        nc.vector.memset(ONE, 1.0)
        for s in range(NSEQ):
            if s < NY:
                src = ys_t[s * L : (s + 1) * L, :]
            else:
                src = xs_t[(s - NY) * L : (s - NY + 1) * L, :]
            raw_s = iop.tile([L, D], F32, tag="raw_s", bufs=NSEQ)
            nc.sync.dma_start(out=raw_s, in_=src)
            raw = iop.tile([L, D], F32, tag="raw", bufs=NSEQ)
            nc.vector.tensor_copy(raw, raw_s)
            pst = psp.tile([D, L], F32, tag="pst")
            nc.tensor.transpose(pst, raw, idn)
            nc.scalar.activation(AUG[0:D, s * L : (s + 1) * L], pst, AF.Copy)
            sq = workp.tile([D, L], F32, tag="sq")
            nc.scalar.square(sq, pst)
            nrm = psp.tile([1, L], F32, tag="nrm")
            nc.tensor.matmul(nrm, ones8, sq)
            nc.scalar.activation(
                NRM[0:1, s * L : (s + 1) * L], nrm, AF.Copy, scale=-0.5
            )

        # Flat coefficient tensors, pair-per-partition: index ic*MP + jc.
        C1F = bigp.tile([NP, M * MP], F32)
        RPF = bigp.tile([NP, M * MP], F32)
        # DRAM staging, pair-major: addr = pair*(M*MP) + ic*MP + jc
        c1d = dramp.tile([NP, M * MP], F32, tag="c1d")
        rpd = dramp.tile([NP, M * MP], F32, tag="rpd")

        # ---- Phases B+C per half (16 pairs) to bound SBUF ----
        for h in range(2):
            DIFFB = cbp.tile([M, HP * MP], F32, tag="a")
            # zero pad columns (jc=127 of each 128-block) so coeff math
            # stays finite there (c1=1, r'=1).
            nc.vector.memset(
                DIFFB.rearrange("p (a b) -> p a b", b=MP)[:, :, M : M + 1], 0.0
            )
            for pl in range(HP):
                p = h * HP + pl
                iloc, j = p // NY, p % NY
                xsl = slice((NY + iloc) * L, (NY + iloc + 1) * L)
                ysl = slice(j * L, (j + 1) * L)
                kps = psp.tile([L, L], F32, tag="kps")
                nc.tensor.matmul(
                    kps, AUG[:, xsl], AUG[:, ysl], start=True, stop=False
                )
                nc.tensor.matmul(
                    kps, NRM[:, xsl], ONE, start=False, stop=False
                )
                nc.tensor.matmul(
                    kps, ONE, NRM[:, ysl], start=False, stop=True
                )
                kex = workp.tile([L, L], F32, tag="kex")
                nc.scalar.activation(kex, kps, AF.Exp)
                # column diff along free dim
                db = workp.tile([L, M], F32, tag="db")
                nc.vector.tensor_sub(db, kex[:, 1:L], kex[:, 0:M])
                # row shift via PE: dbs[a,:] = db[a+1,:]
                dbs = psp.tile([L, M], F32, tag="dbs")
                nc.tensor.matmul(dbs, shf, db)
                nc.vector.tensor_sub(
                    DIFFB[:, pl * MP : pl * MP + M], dbs[0:M, :], db[0:M, :]
                )
            # coefficient build (bulk) on [127, 16*128]:
            #   QB = DIFFB^2 ; T1 = QB/192 + 1
            #   c1c = DIFFB/8 + T1 ; c2c = 2 - T1 ; rpc = c2c / c1c
            QB = cbp.tile([M, HP * MP], F32, tag="b")
            nc.scalar.square(QB, DIFFB)
            T1 = cbp.tile([M, HP * MP], F32, tag="c")
            nc.scalar.activation(T1, QB, AF.Copy, bias=1.0, scale=1.0 / 192.0)
            c1c = cbp.tile([M, HP * MP], F32, tag="b")
            nc.vector.scalar_tensor_tensor(c1c, DIFFB, 0.125, T1, AL.mult, AL.add)
            c2c = cbp.tile([M, HP * MP], F32, tag="a")
            nc.scalar.activation(c2c, T1, AF.Copy, bias=2.0, scale=-1.0)
            # exact reciprocal: approx_fast's one-sided NR bias (~3e-6)
            # accumulates over all 64k PDE cells into an O(0.2) output error.
            ic1 = cbp.tile([M, HP * MP], F32, tag="c")
            nc.vector.reciprocal(out=ic1, in_=c1c)
            rpc = cbp.tile([M, HP * MP], F32, tag="d")
            nc.vector.tensor_mul(rpc, c2c, ic1)
            # store pair-major: DRAM view [ic, pair, jc]; SBUF view
            # [ic, pl, jc]. 128-elem (512B) contiguous runs.
            for dr, sb in ((c1d, c1c), (rpd, rpc)):
                drv = dr.rearrange("p (i j) -> i p j", j=MP)
                nc.scalar.dma_start(
                    out=drv[:, h * HP : (h + 1) * HP, :],
                    in_=sb.rearrange("p (a b) -> p a b", b=MP),
                )

        # ---- Phase D: reload in [pair, ic*MP+jc] layout (2 chunks each) ----
        for dst, src_d in ((C1F, c1d), (RPF, rpd)):
            nc.sync.dma_start(out=dst[:, :], in_=src_d[:, :])

        # ---- Phase E: 254 fused row sweeps, all on DVE ----
        KA = constp.tile([NP, G + 1], F32)
        KB = constp.tile([NP, G + 1], F32)
        nc.vector.memset(KA[:, :], 1.0)    # grid row 0 = 1
        nc.vector.memset(KB[:, 0:1], 1.0)  # j=0 boundary

        cur, prv = KB, KA
        c1rep = None
        for i in range(1, G + 1):
            ic = (i - 1) // 2
            csl = slice(ic * MP, ic * MP + M)
            if i % 2 == 1:
                # expand c1 row to the fine grid once per coarse row (ACT,
                # runs ahead of the DVE sweep; scan data1 must be 2-D).
                c1rep = c1wp.tile([NP, G], F32, tag="c1rep", bufs=8)
                nc.scalar.activation(
                    c1rep.rearrange("p (a b) -> p a b", b=2),
                    _rep2(C1F[:, csl]),
                    AF.Copy,
                )
            # u_j = prv_j - r'_j * prv_{j-1}
            m = rowp.tile([NP, G], F32, tag="m")
            nc.vector.tensor_mul(
                m.rearrange("p (a b) -> p a b", b=2),
                _rep2(RPF[:, csl]),
                prv[:, 0:G].rearrange("p (a b) -> p a b", b=2),
            )
            u = rowp.tile([NP, G], F32, tag="u")
            nc.vector.tensor_sub(u, prv[:, 1 : G + 1], m)
            # y_j = (u_j + y_{j-1}) * c1_j
            nc.vector.tensor_tensor_scan(
                cur[:, 1 : G + 1], u, c1rep, 1.0, AL.add, AL.mult
            )
            cur, prv = prv, cur

        nc.sync.dma_start(out=out_t[:, :], in_=prv[:, G : G + 1])

    nc.finalize()
    return nc


_CACHE = {}


def _get_nc():
    if "nc" not in _CACHE:
        _CACHE["nc"] = _build()
    return _CACHE["nc"]


def run(xs, ys, trace=False):
    xs = np.ascontiguousarray(np.asarray(xs), dtype=np.float32)
    ys = np.ascontiguousarray(np.asarray(ys), dtype=np.float32)
    assert xs.shape == (16, L, D) and ys.shape == (16, L, D)
    nc = _get_nc()
    idn = np.eye(L, dtype=np.float32)
    shf = np.eye(L, k=-1, dtype=np.float32)  # shf[k,m]=1 iff k=m+1
    in_maps = []
    for c in range(N_CORES):
        in_maps.append(
            {
                "xs": xs[2 * c : 2 * c + 2].reshape(NX * L, D).copy(),
                "ys": ys.reshape(NY * L, D).copy(),
                "idn": idn,
                "shf": shf,
            }
        )
    try:
        res = run_bass_kernel_spmd(
            nc, in_maps, list(range(N_CORES)), trace=trace
        )
    except ModuleNotFoundError:
        res = run_bass_kernel_spmd(
            nc, in_maps, list(range(N_CORES)), trace=False
        )
    rows = [res.results[c]["out"].reshape(NX, NY) for c in range(N_CORES)]
    out = np.concatenate(rows, axis=0)
    return out, res


def kernel(xs, ys):
    out, _ = run(xs, ys)
    return out


# revision 14
# speedup vs baseline: 1.3340x; 1.0169x over previous
"""Signature-kernel Gram matrix on 8 NeuronCores.

Math (per pair of sequences x (128,8), y (128,8)):
  K = exp(x@y.T - 0.5|x|^2 - 0.5|y|^2)            (RBF gram, sigma=1)
  diff = second mixed finite difference of K       (127,127)
  Goursat PDE grid G (255,255), G[0,:]=G[:,0]=1,
    G[i,j] = c1*(G[i-1,j]+G[i,j-1]) - c2*G[i-1,j-1]
    c1 = 1 + diff/8 + diff^2/192,  c2 = 1 - diff^2/192
  (dyadic order 1: each coarse cell repeats 2x2 on the 254x254 fine grid)
  answer = G[254,254]

Row-sweep formulation with a single fused scan per fine row:
    y_j = c1_j*(y_{j-1} + u_j),   u_j = prv_j - r'_j*prv_{j-1},
    r' = c2/c1 (precomputed)
-> per row: DVE mult (repeat-view, no materialized coefficient row),
   DVE sub, DVE tensor_tensor_scan(op0=add, op1=mult). All three on DVE,
   no cross-engine dependency on the critical path; ACT expands the
   per-coarse-row c1 multiplier (scan data1 must be a 2-D AP) ahead of
   the sweep.

Coefficients are built bulk in [x-row partitions, pair*col free] layout,
then moved to [pair partitions, row*col free] via one padded pair-major
DRAM round trip (512B-aligned descriptor runs, 4 large DMAs total).

Sharding: data-parallel over batch_x: core c owns x rows {2c, 2c+1} x all
16 ys = 32 pairs. Host gathers the (16,16) output.
"""

import numpy as np
from contextlib import ExitStack

import concourse.bass as bass
import concourse.bacc as bacc
import concourse.tile as tile
from concourse import mybir
from concourse.bass_utils import run_bass_kernel_spmd

F32 = mybir.dt.float32
AL = mybir.AluOpType
AF = mybir.ActivationFunctionType

N_CORES = 8
L = 128          # sequence length
D = 8            # feature dim
NY = 16          # all ys per core
NX = 2           # xs per core
NP = NX * NY     # 32 pairs per core
M = L - 1        # 127 coarse grid
MP = 128         # padded coarse columns (512B DMA runs)
G = 2 * M        # 254 fine grid (dyadic order 1)
HP = NP // 2     # 16 pairs per phase-B/C half


def _rep2(ap):
    """View a [P, n] AP as [P, n, 2] with zero-stride inner dim (each
    element read twice consecutively)."""
    return bass.AP(
        tensor=ap.tensor,
        offset=ap.offset,
        ap=[ap.ap[0], ap.ap[1], [0, 2]],
    )


def _build():
    nc = bacc.Bacc()
    xs_t = nc.dram_tensor("xs", [NX * L, D], F32, kind="ExternalInput")
    ys_t = nc.dram_tensor("ys", [NY * L, D], F32, kind="ExternalInput")
    idn_t = nc.dram_tensor("idn", [L, L], F32, kind="ExternalInput")
    shf_t = nc.dram_tensor("shf", [L, L], F32, kind="ExternalInput")
    out_t = nc.dram_tensor("out", [NP, 1], F32, kind="ExternalOutput")

    NSEQ = NX + NY

    with ExitStack() as ctx:
        tc = ctx.enter_context(tile.TileContext(nc))
        constp = ctx.enter_context(tc.tile_pool(name="constp", bufs=1))
        iop = ctx.enter_context(tc.tile_pool(name="iop", bufs=3))
        psp = ctx.enter_context(tc.tile_pool(name="psp", bufs=2, space="PSUM"))
        workp = ctx.enter_context(tc.tile_pool(name="workp", bufs=3))
        cbp = ctx.enter_context(tc.tile_pool(name="cbp", bufs=1))
        bigp = ctx.enter_context(tc.tile_pool(name="bigp", bufs=1))
        rowp = ctx.enter_context(tc.tile_pool(name="rowp", bufs=4))
        c1wp = ctx.enter_context(tc.tile_pool(name="c1wp", bufs=1))
        dramp = ctx.enter_context(tc.tile_pool(name="dramp", bufs=1, space="DRAM"))

        # Stage DMA-loaded constants through a DVE copy so PE matmuls never
        # wait directly on DMA-queue semaphores (codegen rejects a PE op
        # with two DMA-HW waits).
        idn_s = iop.tile([L, L], F32, tag="idn_s", bufs=1)
        nc.sync.dma_start(out=idn_s, in_=idn_t[:, :])
        idn = constp.tile([L, L], F32)
        nc.vector.tensor_copy(idn, idn_s)
        shf_s = iop.tile([L, L], F32, tag="shf_s", bufs=1)
        nc.sync.dma_start(out=shf_s, in_=shf_t[:, :])
        shf = constp.tile([L, L], F32)
        nc.vector.tensor_copy(shf, shf_s)
        ones8 = constp.tile([D, 1], F32)
        nc.vector.memset(ones8, 1.0)

        # ---- Phase A: transposed sequences + norm rows ----
        AUG = constp.tile([D, NSEQ * L], F32)
        NRM = constp.tile([1, NSEQ * L], F32)
        ONE = constp.tile([1, L], F32)
        nc.vector.memset(ONE, 1.0)
        for s in range(NSEQ):
            if s < NY:
                src = ys_t[s * L : (s + 1) * L, :]
            else:
                src = xs_t[(s - NY) * L : (s - NY + 1) * L, :]
            raw_s = iop.tile([L, D], F32, tag="raw_s", bufs=NSEQ)
            nc.sync.dma_start(out=raw_s, in_=src)
            raw = iop.tile([L, D], F32, tag="raw", bufs=NSEQ)
            nc.vector.tensor_copy(raw, raw_s)
            pst = psp.tile([D, L], F32, tag="pst")
            nc.tensor.transpose(pst, raw, idn)
            nc.scalar.activation(AUG[0:D, s * L : (s + 1) * L], pst, AF.Copy)
            sq = workp.tile([D, L], F32, tag="sq")
            nc.scalar.square(sq, pst)
            nrm = psp.tile([1, L], F32, tag="nrm")
            nc.tensor.matmul(nrm, ones8, sq)
            nc.scalar.activation(
                NRM[0:1, s * L : (s + 1) * L], nrm, AF.Copy, scale=-0.5
            )

        # Flat coefficient tensors, pair-per-partition: index ic*MP + jc.
        C1F = bigp.tile([NP, M * MP], F32)
        RPF = bigp.tile([NP, M * MP], F32)
        # DRAM staging, pair-major: addr = pair*(M*MP) + ic*MP + jc
        c1d = dramp.tile([NP, M * MP], F32, tag="c1d")
        rpd = dramp.tile([NP, M * MP], F32, tag="rpd")

        # ---- Phases B+C per half (16 pairs) to bound SBUF ----
        # B batches 8 pairs per PSUM group: ys are contiguous in AUG, so one
        # 3-matmul accumulation produces 8 K-matrices side by side, then one
        # exp / one column-diff / one shift-matmul / one row-diff per group.
        BG = 4                       # pairs per phase-B group (matmul out <= 512 elems/bank)
        ONEB = constp.tile([1, BG * L], F32)
        nc.vector.memset(ONEB, 1.0)
        for g in range(NP // BG):
            p0 = g * BG
            iloc, j0 = p0 // NY, p0 % NY
            xsl = slice((NY + iloc) * L, (NY + iloc + 1) * L)
            ysl = slice(j0 * L, (j0 + BG) * L)
            kps = psp.tile([L, BG * L], F32, tag="kps", bufs=1)
            nc.tensor.matmul(
                kps, AUG[:, xsl], AUG[:, ysl], start=True, stop=False
            )
            nc.tensor.matmul(
                kps, NRM[:, xsl], ONEB, start=False, stop=False
            )
            nc.tensor.matmul(
                kps, ONE, NRM[:, ysl], start=False, stop=True
            )
            kex = workp.tile([L, BG * L], F32, tag="kex", bufs=2)
            nc.scalar.activation(kex, kps, AF.Exp)
            # column diff along free dim (per 128-block)
            kv = kex.rearrange("p (a b) -> p a b", b=L)
            db = workp.tile([L, BG * M], F32, tag="db", bufs=2)
            dbv = db.rearrange("p (a b) -> p a b", b=M)
            nc.vector.tensor_sub(dbv, kv[:, :, 1:L], kv[:, :, 0:M])
            # row shift via PE: dbs[a,:] = db[a+1,:]
            dbs = psp.tile([L, BG * M], F32, tag="dbs", bufs=1)
            nc.tensor.matmul(dbs, shf, db)
            DIFFB = cbp.tile([M, BG * MP], F32, tag="a")
            # zero pad columns (jc=127 of each 128-block) so coeff math
            # stays finite there (c1=1, r'=1).
            nc.vector.memset(
                DIFFB.rearrange("p (a b) -> p a b", b=MP)[:, :, M : M + 1], 0.0
            )
            dfv = DIFFB.rearrange("p (a b) -> p a b", b=MP)
            nc.vector.tensor_sub(
                dfv[:, :, 0:M],
                dbs[0:M, :].rearrange("p (a b) -> p a b", b=M),
                db[0:M, :].rearrange("p (a b) -> p a b", b=M),
            )
            # coefficient build (bulk) on [127, 8*128]:
            #   QB = DIFFB^2 ; T1 = QB/192 + 1
            #   c1c = DIFFB/8 + T1 ; c2c = 2 - T1 ; rpc = c2c / c1c
            QB = cbp.tile([M, BG * MP], F32, tag="b")
            nc.scalar.square(QB, DIFFB)
            T1 = cbp.tile([M, BG * MP], F32, tag="c")
            nc.scalar.activation(T1, QB, AF.Copy, bias=1.0, scale=1.0 / 192.0)
            c1c = cbp.tile([M, BG * MP], F32, tag="b")
            nc.vector.scalar_tensor_tensor(c1c, DIFFB, 0.125, T1, AL.mult, AL.add)
            c2c = cbp.tile([M, BG * MP], F32, tag="a")
            nc.scalar.activation(c2c, T1, AF.Copy, bias=2.0, scale=-1.0)
            # exact reciprocal: approx_fast's one-sided NR bias (~3e-6)
            # accumulates over all 64k PDE cells into an O(0.2) output error.
            ic1 = cbp.tile([M, BG * MP], F32, tag="c")
            nc.vector.reciprocal(out=ic1, in_=c1c)
            rpc = cbp.tile([M, BG * MP], F32, tag="d")
            nc.vector.tensor_mul(rpc, c2c, ic1)
            # store pair-major: DRAM view [ic, pair, jc]; SBUF view
            # [ic, pl, jc]. 128-elem (512B) contiguous runs.
            for dr, sb in ((c1d, c1c), (rpd, rpc)):
                drv = dr.rearrange("p (i j) -> i p j", j=MP)
                nc.scalar.dma_start(
                    out=drv[:, g * BG : (g + 1) * BG, :],
                    in_=sb.rearrange("p (a b) -> p a b", b=MP),
                )

        # ---- Phase D: reload in [pair, ic*MP+jc] layout (2 chunks each) ----
        for dst, src_d in ((C1F, c1d), (RPF, rpd)):
            nc.sync.dma_start(out=dst[:, :], in_=src_d[:, :])

        # ---- Phase E: 254 fused row sweeps, all on DVE ----
        KA = constp.tile([NP, G + 1], F32)
        KB = constp.tile([NP, G + 1], F32)
        nc.vector.memset(KA[:, :], 1.0)    # grid row 0 = 1
        nc.vector.memset(KB[:, 0:1], 1.0)  # j=0 boundary

        cur, prv = KB, KA
        c1rep = None
        for i in range(1, G + 1):
            ic = (i - 1) // 2
            csl = slice(ic * MP, ic * MP + M)
            if i % 2 == 1:
                # expand c1 row to the fine grid once per coarse row (ACT,
                # runs ahead of the DVE sweep; scan data1 must be 2-D).
                c1rep = c1wp.tile([NP, G], F32, tag="c1rep", bufs=5)
                nc.scalar.activation(
                    c1rep.rearrange("p (a b) -> p a b", b=2),
                    _rep2(C1F[:, csl]),
                    AF.Copy,
                )
            # u_j = prv_j - r'_j * prv_{j-1}
            m = rowp.tile([NP, G], F32, tag="m", bufs=2)
            nc.vector.tensor_mul(
                m.rearrange("p (a b) -> p a b", b=2),
                _rep2(RPF[:, csl]),
                prv[:, 0:G].rearrange("p (a b) -> p a b", b=2),
            )
            u = rowp.tile([NP, G], F32, tag="u", bufs=2)
            nc.vector.tensor_sub(u, prv[:, 1 : G + 1], m)
            # y_j = (u_j + y_{j-1}) * c1_j
            nc.vector.tensor_tensor_scan(
                cur[:, 1 : G + 1], u, c1rep, 1.0, AL.add, AL.mult
            )
            cur, prv = prv, cur

        nc.sync.dma_start(out=out_t[:, :], in_=prv[:, G : G + 1])

    nc.finalize()
    return nc


_CACHE = {}


def _get_nc():
    if "nc" not in _CACHE:
        _CACHE["nc"] = _build()
    return _CACHE["nc"]


def run(xs, ys, trace=False):
    xs = np.ascontiguousarray(np.asarray(xs), dtype=np.float32)
    ys = np.ascontiguousarray(np.asarray(ys), dtype=np.float32)
    assert xs.shape == (16, L, D) and ys.shape == (16, L, D)
    nc = _get_nc()
    idn = np.eye(L, dtype=np.float32)
    shf = np.eye(L, k=-1, dtype=np.float32)  # shf[k,m]=1 iff k=m+1
    in_maps = []
    for c in range(N_CORES):
        in_maps.append(
            {
                "xs": xs[2 * c : 2 * c + 2].reshape(NX * L, D).copy(),
                "ys": ys.reshape(NY * L, D).copy(),
                "idn": idn,
                "shf": shf,
            }
        )
    try:
        res = run_bass_kernel_spmd(
            nc, in_maps, list(range(N_CORES)), trace=trace
        )
    except ModuleNotFoundError:
        res = run_bass_kernel_spmd(
            nc, in_maps, list(range(N_CORES)), trace=False
        )
    rows = [res.results[c]["out"].reshape(NX, NY) for c in range(N_CORES)]
    out = np.concatenate(rows, axis=0)
    return out, res


def kernel(xs, ys):
    out, _ = run(xs, ys)
    return out


# revision 15
# speedup vs baseline: 1.3614x; 1.0205x over previous
"""Signature-kernel Gram matrix on 8 NeuronCores.

Math (per pair of sequences x (128,8), y (128,8)):
  K = exp(x@y.T - 0.5|x|^2 - 0.5|y|^2)            (RBF gram, sigma=1)
  diff = second mixed finite difference of K       (127,127)
  Goursat PDE grid G (255,255), G[0,:]=G[:,0]=1,
    G[i,j] = c1*(G[i-1,j]+G[i,j-1]) - c2*G[i-1,j-1]
    c1 = 1 + diff/8 + diff^2/192,  c2 = 1 - diff^2/192
  (dyadic order 1: each coarse cell repeats 2x2 on the 254x254 fine grid)
  answer = G[254,254]

Row-sweep formulation with a single fused scan per fine row:
    y_j = c1_j*(y_{j-1} + u_j),   u_j = prv_j - r'_j*prv_{j-1},
    r' = c2/c1 (precomputed)
-> per row: DVE mult (repeat-view, no materialized coefficient row),
   DVE sub, DVE tensor_tensor_scan(op0=add, op1=mult). All three on DVE,
   no cross-engine dependency on the critical path; ACT expands the
   per-coarse-row c1 multiplier (scan data1 must be a 2-D AP) ahead of
   the sweep.

Coefficients are built bulk in [x-row partitions, pair*col free] layout,
then moved to [pair partitions, row*col free] via one padded pair-major
DRAM round trip (512B-aligned descriptor runs, 4 large DMAs total).

Sharding: data-parallel over batch_x: core c owns x rows {2c, 2c+1} x all
16 ys = 32 pairs. Host gathers the (16,16) output.
"""

import numpy as np
from contextlib import ExitStack

import concourse.bass as bass
import concourse.bacc as bacc
import concourse.tile as tile
from concourse import mybir
from concourse.bass_utils import run_bass_kernel_spmd

F32 = mybir.dt.float32
AL = mybir.AluOpType
AF = mybir.ActivationFunctionType

N_CORES = 8
L = 128          # sequence length
D = 8            # feature dim
NY = 16          # all ys per core
NX = 2           # xs per core
NP = NX * NY     # 32 pairs per core
M = L - 1        # 127 coarse grid
MP = 128         # padded coarse columns (512B DMA runs)
G = 2 * M        # 254 fine grid (dyadic order 1)
HP = NP // 2     # 16 pairs per phase-B/C half


def _rep2(ap):
    """View a [P, n] AP as [P, n, 2] with zero-stride inner dim (each
    element read twice consecutively)."""
    return bass.AP(
        tensor=ap.tensor,
        offset=ap.offset,
        ap=[ap.ap[0], ap.ap[1], [0, 2]],
    )


def _build():
    nc = bacc.Bacc()
    xs_t = nc.dram_tensor("xs", [NX * L, D], F32, kind="ExternalInput")
    ys_t = nc.dram_tensor("ys", [NY * L, D], F32, kind="ExternalInput")
    idn_t = nc.dram_tensor("idn", [L, L], F32, kind="ExternalInput")
    shf_t = nc.dram_tensor("shf", [L, L], F32, kind="ExternalInput")
    out_t = nc.dram_tensor("out", [NP, 1], F32, kind="ExternalOutput")

    NSEQ = NX + NY

    with ExitStack() as ctx:
        tc = ctx.enter_context(tile.TileContext(nc))
        constp = ctx.enter_context(tc.tile_pool(name="constp", bufs=1))
        iop = ctx.enter_context(tc.tile_pool(name="iop", bufs=3))
        psp = ctx.enter_context(tc.tile_pool(name="psp", bufs=2, space="PSUM"))
        workp = ctx.enter_context(tc.tile_pool(name="workp", bufs=3))
        cbp = ctx.enter_context(tc.tile_pool(name="cbp", bufs=1))
        bigp = ctx.enter_context(tc.tile_pool(name="bigp", bufs=1))
        rowp = ctx.enter_context(tc.tile_pool(name="rowp", bufs=4))
        c1wp = ctx.enter_context(tc.tile_pool(name="c1wp", bufs=1))
        dramp = ctx.enter_context(tc.tile_pool(name="dramp", bufs=1, space="DRAM"))

        # Stage DMA-loaded constants through a DVE copy so PE matmuls never
        # wait directly on DMA-queue semaphores (codegen rejects a PE op
        # with two DMA-HW waits).
        idn_s = iop.tile([L, L], F32, tag="idn_s", bufs=1)
        nc.sync.dma_start(out=idn_s, in_=idn_t[:, :])
        idn = constp.tile([L, L], F32)
        nc.vector.tensor_copy(idn, idn_s)
        shf_s = iop.tile([L, L], F32, tag="shf_s", bufs=1)
        nc.sync.dma_start(out=shf_s, in_=shf_t[:, :])
        shf = constp.tile([L, L], F32)
        nc.vector.tensor_copy(shf, shf_s)
        ones8 = constp.tile([D, 1], F32)
        nc.vector.memset(ones8, 1.0)

        # ---- Phase A: transposed sequences + norm rows ----
        AUG = constp.tile([D, NSEQ * L], F32)
        NRM = constp.tile([1, NSEQ * L], F32)
        ONE = constp.tile([1, L], F32)
        nc.vector.memset(ONE, 1.0)
        for s in range(NSEQ):
            if s < NY:
                src = ys_t[s * L : (s + 1) * L, :]
            else:
                src = xs_t[(s - NY) * L : (s - NY + 1) * L, :]
            raw_s = iop.tile([L, D], F32, tag="raw_s", bufs=NSEQ)
            nc.sync.dma_start(out=raw_s, in_=src)
            raw = iop.tile([L, D], F32, tag="raw", bufs=NSEQ)
            nc.vector.tensor_copy(raw, raw_s)
            pst = psp.tile([D, L], F32, tag="pst")
            nc.tensor.transpose(pst, raw, idn)
            nc.scalar.activation(AUG[0:D, s * L : (s + 1) * L], pst, AF.Copy)
            sq = workp.tile([D, L], F32, tag="sq")
            nc.scalar.square(sq, pst)
            nrm = psp.tile([1, L], F32, tag="nrm")
            nc.tensor.matmul(nrm, ones8, sq)
            nc.scalar.activation(
                NRM[0:1, s * L : (s + 1) * L], nrm, AF.Copy, scale=-0.5
            )

        # Flat coefficient tensors, pair-per-partition: index ic*MP + jc.
        C1F = bigp.tile([NP, M * MP], F32)
        RPF = bigp.tile([NP, M * MP], F32)
        # DRAM staging, pair-major: addr = pair*(M*MP) + ic*MP + jc
        c1d = dramp.tile([NP, M * MP], F32, tag="c1d")
        rpd = dramp.tile([NP, M * MP], F32, tag="rpd")

        # ---- Phases B+C per half (16 pairs) to bound SBUF ----
        # B batches 8 pairs per PSUM group: ys are contiguous in AUG, so one
        # 3-matmul accumulation produces 8 K-matrices side by side, then one
        # exp / one column-diff / one shift-matmul / one row-diff per group.
        BG = 4                       # pairs per phase-B group (matmul out <= 512 elems/bank)
        ONEB = constp.tile([1, BG * L], F32)
        nc.vector.memset(ONEB, 1.0)
        for g in range(NP // BG):
            p0 = g * BG
            iloc, j0 = p0 // NY, p0 % NY
            xsl = slice((NY + iloc) * L, (NY + iloc + 1) * L)
            ysl = slice(j0 * L, (j0 + BG) * L)
            kps = psp.tile([L, BG * L], F32, tag="kps", bufs=2)
            nc.tensor.matmul(
                kps, AUG[:, xsl], AUG[:, ysl], start=True, stop=False
            )
            nc.tensor.matmul(
                kps, NRM[:, xsl], ONEB, start=False, stop=False
            )
            nc.tensor.matmul(
                kps, ONE, NRM[:, ysl], start=False, stop=True
            )
            kex = workp.tile([L, BG * L], F32, tag="kex", bufs=2)
            nc.scalar.activation(kex, kps, AF.Exp)
            # column diff along free dim (per 128-block)
            kv = kex.rearrange("p (a b) -> p a b", b=L)
            db = workp.tile([L, BG * M], F32, tag="db", bufs=2)
            dbv = db.rearrange("p (a b) -> p a b", b=M)
            nc.vector.tensor_sub(dbv, kv[:, :, 1:L], kv[:, :, 0:M])
            # row shift via PE: dbs[a,:] = db[a+1,:]
            dbs = psp.tile([L, BG * M], F32, tag="dbs", bufs=2)
            nc.tensor.matmul(dbs, shf, db)
            DIFFB = cbp.tile([M, BG * MP], F32, tag="a", bufs=2)
            # zero pad columns (jc=127 of each 128-block) so coeff math
            # stays finite there (c1=1, r'=1).
            nc.vector.memset(
                DIFFB.rearrange("p (a b) -> p a b", b=MP)[:, :, M : M + 1], 0.0
            )
            dfv = DIFFB.rearrange("p (a b) -> p a b", b=MP)
            nc.vector.tensor_sub(
                dfv[:, :, 0:M],
                dbs[0:M, :].rearrange("p (a b) -> p a b", b=M),
                db[0:M, :].rearrange("p (a b) -> p a b", b=M),
            )
            # coefficient build (bulk) on [127, 8*128]:
            #   QB = DIFFB^2 ; T1 = QB/192 + 1
            #   c1c = DIFFB/8 + T1 ; c2c = 2 - T1 ; rpc = c2c / c1c
            QB = cbp.tile([M, BG * MP], F32, tag="b", bufs=2)
            nc.scalar.square(QB, DIFFB)
            T1 = cbp.tile([M, BG * MP], F32, tag="c", bufs=2)
            nc.scalar.activation(T1, QB, AF.Copy, bias=1.0, scale=1.0 / 192.0)
            c1c = cbp.tile([M, BG * MP], F32, tag="b", bufs=2)
            nc.vector.scalar_tensor_tensor(c1c, DIFFB, 0.125, T1, AL.mult, AL.add)
            c2c = cbp.tile([M, BG * MP], F32, tag="a", bufs=2)
            nc.scalar.activation(c2c, T1, AF.Copy, bias=2.0, scale=-1.0)
            # exact reciprocal: approx_fast's one-sided NR bias (~3e-6)
            # accumulates over all 64k PDE cells into an O(0.2) output error.
            ic1 = cbp.tile([M, BG * MP], F32, tag="c", bufs=2)
            nc.vector.reciprocal(out=ic1, in_=c1c)
            rpc = cbp.tile([M, BG * MP], F32, tag="d", bufs=2)
            nc.vector.tensor_mul(rpc, c2c, ic1)
            # store pair-major: DRAM view [ic, pair, jc]; SBUF view
            # [ic, pl, jc]. 128-elem (512B) contiguous runs.
            for dr, sb in ((c1d, c1c), (rpd, rpc)):
                drv = dr.rearrange("p (i j) -> i p j", j=MP)
                nc.scalar.dma_start(
                    out=drv[:, g * BG : (g + 1) * BG, :],
                    in_=sb.rearrange("p (a b) -> p a b", b=MP),
                )

        # ---- Phase D: reload in [pair, ic*MP+jc] layout (2 chunks each) ----
        for dst, src_d in ((C1F, c1d), (RPF, rpd)):
            nc.sync.dma_start(out=dst[:, :], in_=src_d[:, :])

        # ---- Phase E: 254 fused row sweeps, all on DVE ----
        KA = constp.tile([NP, G + 1], F32)
        KB = constp.tile([NP, G + 1], F32)
        nc.vector.memset(KA[:, :], 1.0)    # grid row 0 = 1
        nc.vector.memset(KB[:, 0:1], 1.0)  # j=0 boundary

        cur, prv = KB, KA
        c1rep = None
        for i in range(1, G + 1):
            ic = (i - 1) // 2
            csl = slice(ic * MP, ic * MP + M)
            if i % 2 == 1:
                # expand c1 row to the fine grid once per coarse row (ACT,
                # runs ahead of the DVE sweep; scan data1 must be 2-D).
                c1rep = c1wp.tile([NP, G], F32, tag="c1rep", bufs=5)
                nc.scalar.activation(
                    c1rep.rearrange("p (a b) -> p a b", b=2),
                    _rep2(C1F[:, csl]),
                    AF.Copy,
                )
            # u_j = prv_j - r'_j * prv_{j-1}
            m = rowp.tile([NP, G], F32, tag="m", bufs=2)
            nc.vector.tensor_mul(
                m.rearrange("p (a b) -> p a b", b=2),
                _rep2(RPF[:, csl]),
                prv[:, 0:G].rearrange("p (a b) -> p a b", b=2),
            )
            u = rowp.tile([NP, G], F32, tag="u", bufs=2)
            nc.vector.tensor_sub(u, prv[:, 1 : G + 1], m)
            # y_j = (u_j + y_{j-1}) * c1_j
            nc.vector.tensor_tensor_scan(
                cur[:, 1 : G + 1], u, c1rep, 1.0, AL.add, AL.mult
            )
            cur, prv = prv, cur

        nc.sync.dma_start(out=out_t[:, :], in_=prv[:, G : G + 1])

    nc.finalize()
    return nc


_CACHE = {}


def _get_nc():
    if "nc" not in _CACHE:
        _CACHE["nc"] = _build()
    return _CACHE["nc"]


def run(xs, ys, trace=False):
    xs = np.ascontiguousarray(np.asarray(xs), dtype=np.float32)
    ys = np.ascontiguousarray(np.asarray(ys), dtype=np.float32)
    assert xs.shape == (16, L, D) and ys.shape == (16, L, D)
    nc = _get_nc()
    idn = np.eye(L, dtype=np.float32)
    shf = np.eye(L, k=-1, dtype=np.float32)  # shf[k,m]=1 iff k=m+1
    in_maps = []
    for c in range(N_CORES):
        in_maps.append(
            {
                "xs": xs[2 * c : 2 * c + 2].reshape(NX * L, D).copy(),
                "ys": ys.reshape(NY * L, D).copy(),
                "idn": idn,
                "shf": shf,
            }
        )
    try:
        res = run_bass_kernel_spmd(
            nc, in_maps, list(range(N_CORES)), trace=trace
        )
    except ModuleNotFoundError:
        res = run_bass_kernel_spmd(
            nc, in_maps, list(range(N_CORES)), trace=False
        )
    rows = [res.results[c]["out"].reshape(NX, NY) for c in range(N_CORES)]
    out = np.concatenate(rows, axis=0)
    return out, res


def kernel(xs, ys):
    out, _ = run(xs, ys)
    return out
